# revision 3
# baseline (speedup 1.0000x reference)
"""HausdorffDT loss kernel for Trainium2 (8 NeuronCores, Bass/Tile).

Math: with ALPHA=2 and field(m) = sqrt(edt2(m)) + sqrt(edt2(~m)), one of the
two terms is zero at every pixel, so field(m)^2 == edt2(m) + edt2(~m) exactly.
The loss is therefore

    mean( (x - onehot)^2 * (edt2(pm)+edt2(~pm) + edt2(tm)+edt2(~tm)) )

with an all-zero-field guard per empty mask.  Squared EDTs are small exact
integers, so the distance pipeline runs in bf16 exactly:

  1. row pass: exact 1D distance to nearest True along W via two chained
     tensor_tensor_scan min-plus recurrences (fwd + bwd), batched over all
     fields with INF padding between row segments.
  2. transpose the row distances via the (otherwise idle) tensor engine,
     squaring them in the PSUM->SBUF drain on the scalar engine (fused).
  3. column pass: windowed parabola min-plus
     acc = min(acc, g[i +- d] + d^2), d = 1..R.  Exact whenever the true max
     distance of the mask is <= R, which is validated on-device: row pads of
     width PAD with PAD^2 > R^2 guarantee every corrupted/leaked row distance
     contributes candidates > R^2, so  max(final acc) <= R^2  iff the result
     is the exact EDT^2 (see VALIDATION below).
  4. weighted reduce against (x - onehot)^2 directly in the transposed
     domain (the weights are transposed instead of the fields: 16 tiles
     instead of 32), fp32 accumulation per (kind, class) plus a per-mask max
     for validation; host applies empty-mask guards + mean.

VALIDATION: acc >= true EDT^2 always (windowing only drops candidates;
corrupted row distances are >= PAD > R).  If max(acc per live mask) <= R^2,
every pixel's optimal candidate (|di| <= dist <= R, row dist <= R < PAD) was
in the window with its exact value, so acc == EDT^2 exactly.  Otherwise the
host escalates to an R=15 build, then to an exact numpy fallback.

Sharding: data-parallel over batch, one sample per core; per-partition
partial sums are combined on the host (scalar loss; no collectives).
"""

import numpy as np

B, C, H, W = 8, 4, 256, 256
NCORES = 8
P = 128
INF = 4096.0
R_PRED, R_TGT = 4, 5     # primary window radii (validated on device)
R_MAX = 15               # escalation build; beyond that -> exact numpy

_CACHE = {}
LAST_RESULT = None        # raw per-core outputs of the most recent device run
LAST_EXEC_WALL_NS = None  # wall-clock of the device dispatch+exec call


def _seg(k, t, c, j):
    """Segment index: kind, polarity, class, chunk (h in row dom, v in col)."""
    return ((k * 2 + t) * 4 + c) * 2 + j


# --------------------------------------------------------------- bass kernel

def _build(R_pred, R_tgt, pad):
    import concourse.bacc as bacc
    import concourse.mybir as mybir
    from concourse.tile import TileContext

    dt = mybir.dt
    op = mybir.AluOpType
    seg = pad + W + pad
    free = 32 * seg
    Rk = {0: R_pred, 1: R_tgt}

    nc = bacc.Bacc("TRN2", target_bir_lowering=False, debug=False,
                   enable_asserts=False, num_devices=NCORES)
    xb = nc.dram_tensor("x", [C, H, W], dt.float16, kind="ExternalInput")
    yb = nc.dram_tensor("y", [H, W], dt.int32, kind="ExternalInput")
    eb = nc.dram_tensor("eye", [P, P], dt.bfloat16, kind="ExternalInput")
    ob = nc.dram_tensor("out", [P, 16], dt.float32, kind="ExternalOutput")

    with TileContext(nc) as tc:
        with tc.tile_pool(name="main", bufs=1) as pool, \
             tc.tile_pool(name="ps", bufs=8, space="PSUM") as psp:
            x_sb = pool.tile([P, C * 2 * W], dt.float16, tag="x_sb")
            y_sb = pool.tile([P, 2 * W], dt.int32, tag="y_sb")
            eye = pool.tile([P, P], dt.bfloat16, tag="eye")
            m_tgt = pool.tile([P, C * 2 * W], dt.bfloat16, tag="m_tgt")
            a = pool.tile([P, free], dt.bfloat16, tag="a")
            f = pool.tile([P, free], dt.bfloat16, tag="f")
            ones = pool.tile([P, free], dt.bfloat16, tag="ones")
            gT = pool.tile([P, free], dt.bfloat16, tag="gT")
            gT1 = pool.tile([P, free], dt.bfloat16, tag="gT1")
            acc = pool.tile([P, 32 * W], dt.bfloat16, tag="acc")
            tdiff = pool.tile([P, C * 2 * W], dt.bfloat16, tag="tdiff")
            wsqT = pool.tile([P, C * 2 * W], dt.bfloat16, tag="wsqT")
            junk = pool.tile([P, 2 * W], dt.bfloat16, tag="junk")
            fin = pool.tile([P, 16], dt.float32, tag="fin")

            # ---- loads
            nc.sync.dma_start(
                out=x_sb[:, :].rearrange("p (c hh w) -> p c hh w", c=C, hh=2),
                in_=xb.ap().rearrange("c (hh p) w -> p c hh w", p=P))
            nc.sync.dma_start(
                out=y_sb[:, :].rearrange("p (hh w) -> p hh w", hh=2),
                in_=yb.ap().rearrange("(hh p) w -> p hh w", p=P))
            nc.sync.dma_start(out=eye[:, :], in_=eb.ap())

            nc.vector.memset(a[:, :], INF)
            nc.vector.memset(ones[:, :], 1.0)
            nc.vector.memset(gT[:, :], INF)  # squared-domain pad value

            # ---- target one-hot masks (bf16 0/1), layout matches x_sb
            for c in range(C):
                nc.vector.tensor_scalar(
                    out=m_tgt[:, c * 2 * W:(c + 1) * 2 * W],
                    in0=y_sb[:, :], scalar1=float(c), scalar2=None,
                    op0=op.is_equal)

            def live8(tile, k, t):
                """[p, 8, W] live columns of the 8 row segments of (k, t)."""
                base = _seg(k, t, 0, 0) * seg
                v = tile[:, base:base + 8 * seg]
                v = v.rearrange("p (s w) -> p s w", w=seg)
                return v[:, :, pad:pad + W]

            x8 = x_sb[:, :].rearrange("p (s w) -> p s w", w=W)
            m8 = m_tgt[:, :].rearrange("p (s w) -> p s w", w=W)

            # ---- scan input a: 0 at zero-set pixels, INF elsewhere
            nc.vector.tensor_scalar(out=live8(a, 0, 0), in0=x8, scalar1=0.5,
                                    scalar2=INF, op0=op.is_le, op1=op.mult)
            nc.vector.tensor_scalar(out=live8(a, 0, 1), in0=x8, scalar1=0.5,
                                    scalar2=INF, op0=op.is_gt, op1=op.mult)
            nc.vector.tensor_scalar(out=live8(a, 1, 0), in0=m8, scalar1=-INF,
                                    scalar2=INF, op0=op.mult, op1=op.add)
            nc.vector.tensor_scalar(out=live8(a, 1, 1), in0=m8, scalar1=INF,
                                    scalar2=None, op0=op.mult)

            # ---- row pass: two chained min-plus scans per (kind, pol) block
            for q in range(4):
                s0, s1 = q * 8 * seg, (q + 1) * 8 * seg
                nc.vector.tensor_tensor_scan(
                    out=f[:, s0:s1], data0=ones[:, s0:s1], data1=a[:, s0:s1],
                    initial=INF, op0=op.add, op1=op.min)
                nc.vector.tensor_tensor_scan(
                    out=a[:, s0:s1][:, ::-1], data0=ones[:, s0:s1][:, ::-1],
                    data1=f[:, s0:s1][:, ::-1],
                    initial=INF, op0=op.add, op1=op.min)

            # ---- transpose d1 via PE; square on the PSUM->SBUF drain
            for k in range(2):
                for t in range(2):
                    for c in range(C):
                        for h in range(2):
                            for v in range(2):
                                ps = psp.tile([P, P], dt.bfloat16,
                                              name="ps", tag="ps")
                                sb = _seg(k, t, c, h) * seg + pad + 128 * v
                                nc.tensor.transpose(
                                    ps[:, :], a[:, sb:sb + 128], eye[:, :])
                                db = _seg(k, t, c, v) * seg + pad + 128 * h
                                nc.scalar.square(out=gT[:, db:db + 128],
                                                 in_=ps[:, :])
                # odd-shift alias for DVE 2x alignment (scalar engine copy)
                kb, ke = k * 16 * seg, (k + 1) * 16 * seg
                nc.scalar.copy(out=gT1[:, kb:ke - 1], in_=gT[:, kb + 1:ke])

            # ---- column pass: acc = min_d ( g[i +- d] + d^2 ), d = 0..Rk
            for k in range(2):
                accv = acc[:, k * 16 * W:(k + 1) * 16 * W]
                accv = accv.rearrange("p (s w) -> p s w", w=W)

                def gview(tile, shift, k=k):
                    vv = tile[:, k * 16 * seg:(k + 1) * 16 * seg]
                    vv = vv.rearrange("p (s w) -> p s w", w=seg)
                    return vv[:, :, pad + shift:pad + shift + W]

                first = True
                ds = [d for d in range(2, Rk[k] + 1, 2)] + \
                     [d for d in range(1, Rk[k] + 1, 2)]
                for d in ds:
                    for sgn in (1, -1):
                        if d % 2 == 0:
                            in0 = gview(gT, sgn * d)
                        else:
                            in0 = gview(gT1, sgn * d - 1)
                        in1 = gview(gT, 0) if first else accv
                        nc.vector.scalar_tensor_tensor(
                            out=accv, in0=in0, scalar=float(d * d), in1=in1,
                            op0=op.add, op1=op.min)
                        first = False

            # ---- S = edt2(m) + edt2(~m): pol T += pol F (into T block)
            for k in range(2):
                t0 = acc[:, (k * 16) * W:(k * 16 + 8) * W]
                t1 = acc[:, (k * 16 + 8) * W:(k * 16 + 16) * W]
                nc.vector.tensor_add(out=t0, in0=t0, in1=t1)

            # ---- weights: (x - onehot)^2, transposed via PE + squared drain
            nc.vector.tensor_sub(out=tdiff[:, :], in0=x_sb[:, :],
                                 in1=m_tgt[:, :])
            for c in range(C):
                for h in range(2):
                    for v in range(2):
                        ps = psp.tile([P, P], dt.bfloat16,
                                      name="ps", tag="ps")
                        sb = (c * 2 + h) * W + 128 * v
                        nc.tensor.transpose(
                            ps[:, :], tdiff[:, sb:sb + 128], eye[:, :])
                        db = (c * 2 + v) * W + 128 * h
                        nc.scalar.square(out=wsqT[:, db:db + 128],
                                         in_=ps[:, :])

            # ---- per-(kind,class) weighted sums + validation maxes
            for k in range(2):
                for c in range(C):
                    i = k * 4 + c
                    accs = acc[:, (k * 16 + c * 2) * W:(k * 16 + c * 2 + 2) * W]
                    nc.vector.tensor_tensor_reduce(
                        out=junk[:, :],
                        in0=wsqT[:, c * 2 * W:(c + 1) * 2 * W], in1=accs,
                        scale=1.0, scalar=0.0, op0=op.mult, op1=op.add,
                        accum_out=fin[:, i:i + 1])
                    nc.vector.tensor_reduce(
                        out=fin[:, 8 + i:9 + i], in_=accs,
                        axis=mybir.AxisListType.X, op=op.max)

            nc.sync.dma_start(out=ob.ap(), in_=fin[:, :])

    nc.compile()
    return nc


def _get_nc(key):
    if key not in _CACHE:
        if key == "main":
            _CACHE[key] = _build(R_PRED, R_TGT, 8)
        else:
            _CACHE[key] = _build(R_MAX, R_MAX, 16)
    return _CACHE[key]


def _run_device(nc, x16, y):
    """Run the 8-core SPMD kernel; returns (sums[B,2,C], maxes[B,2,C])."""
    global LAST_RESULT, LAST_EXEC_WALL_NS
    import time
    import ml_dtypes
    from concourse import bass2jax

    eye = np.eye(P, dtype=ml_dtypes.bfloat16)
    in_maps = [{"x": x16[b], "y": y[b], "eye": eye} for b in range(B)]
    t0 = time.perf_counter()
    res = bass2jax.run_bass_via_pjrt(nc, in_maps, n_cores=NCORES)
    LAST_EXEC_WALL_NS = int((time.perf_counter() - t0) * 1e9)
    LAST_RESULT = res
    out = np.stack([res[b]["out"] for b in range(B)])      # [B,128,16]
    sums = out[:, :, :8].astype(np.float64).sum(axis=1).reshape(B, 2, C)
    maxes = out[:, :, 8:].max(axis=1).reshape(B, 2, C)
    return sums, maxes


# ------------------------------------------------------- exact host fallback

def _loss_numpy_exact(x, y):
    BIG = float(H + W)

    def dist1d(z):
        n = z.shape[-1]
        idx = np.arange(n, dtype=np.int64)
        fw = np.maximum.accumulate(np.where(z, idx, -1), axis=-1)
        df = np.where(fw >= 0, (idx - fw).astype(np.float32), np.float32(BIG))
        bw = np.minimum.accumulate(np.where(z, idx, 2 * n)[..., ::-1],
                                   axis=-1)[..., ::-1]
        db = np.where(bw < 2 * n, (bw - idx).astype(np.float32),
                      np.float32(BIG))
        return np.minimum(df, db)

    def edt_sq(z):
        g = dist1d(z).astype(np.float32) ** 2
        i = np.arange(H, dtype=np.float32)
        out = np.empty((H, W), np.float32)
        for i0 in range(0, H, 32):
            off = (i[i0:i0 + 32, None] - i[None, :]) ** 2
            out[i0:i0 + 32] = (off[:, :, None] + g[None, :, :]).min(axis=1)
        return out

    def field(m):
        if not m.any():
            return np.zeros((H, W), np.float32)
        return np.sqrt(edt_sq(~m)) + np.sqrt(edt_sq(m))

    total = 0.0
    for b in range(B):
        for c in range(C):
            oh = (y[b] == c)
            pm = x[b, c] > 0.5
            dist = (field(pm).astype(np.float32) ** 2
                    + field(oh).astype(np.float32) ** 2)
            w = (x[b, c] - oh.astype(np.float32)) ** 2
            total += float((w.astype(np.float64) * dist.astype(np.float64)).sum())
    return np.float32(total / (B * C * H * W))


# ------------------------------------------------------------------- driver

def kernel(x, y):
    x = np.asarray(x, np.float32)
    y = np.ascontiguousarray(np.asarray(y, np.int32))
    assert x.shape == (B, C, H, W) and y.shape == (B, H, W)
    x16 = np.ascontiguousarray(x.astype(np.float16))

    # empty-mask guards (reference zeroes the field of an empty mask)
    g_pred = (x > 0.5).reshape(B, C, -1).any(axis=2)
    g_tgt = np.stack([(y == c).reshape(B, -1).any(axis=1) for c in range(C)],
                     axis=1)
    guards = np.stack([g_pred, g_tgt], axis=1)             # [B,2,C]

    try:
        for key, rr in (("main", (R_PRED, R_TGT)), ("esc", (R_MAX, R_MAX))):
            nc = _get_nc(key)
            sums, maxes = _run_device(nc, x16, y)
            thresh = np.array(rr, np.float32)[None, :, None] ** 2 + 0.5
            if not (maxes <= thresh)[guards].all():
                continue  # window too small for some live mask -> escalate
            total = float((sums * guards).sum())
            return np.asarray(np.float32(total / (B * C * H * W)))
    except Exception as e:  # device unavailable etc. -> exact host fallback
        import sys
        print(f"kernel: device path failed ({type(e).__name__}: {e}); "
              "using exact host fallback", file=sys.stderr)
    return np.asarray(_loss_numpy_exact(x, y))


# revision 4
# speedup vs baseline: 115.6194x; 115.6194x over previous
"""HausdorffDT loss kernel for Trainium2 (8 NeuronCores, Bass/Tile).

Math: with ALPHA=2 and field(m) = sqrt(edt2(m)) + sqrt(edt2(~m)), one of the
two terms is zero at every pixel, so field(m)^2 == edt2(m) + edt2(~m) exactly.
The loss is therefore

    mean( (x - onehot)^2 * (edt2(pm)+edt2(~pm) + edt2(tm)+edt2(~tm)) )

with an all-zero-field guard per empty mask.  Squared EDTs are small exact
integers, so the distance pipeline runs in bf16 exactly:

  1. row pass: exact 1D distance to nearest True along W via two chained
     tensor_tensor_scan min-plus recurrences (fwd + bwd), batched over all
     fields with INF padding between row segments.
  2. transpose the row distances via the (otherwise idle) tensor engine,
     squaring them in the PSUM->SBUF drain on the scalar engine (fused).
  3. column pass: windowed parabola min-plus
     acc = min(acc, g[i +- d] + d^2), d = 1..R.  Exact whenever the true max
     distance of the mask is <= R, which is validated on-device: row pads of
     width PAD with PAD^2 > R^2 guarantee every corrupted/leaked row distance
     contributes candidates > R^2, so  max(final acc) <= R^2  iff the result
     is the exact EDT^2 (see VALIDATION below).
  4. weighted reduce against (x - onehot)^2 directly in the transposed
     domain (the weights are transposed instead of the fields: 16 tiles
     instead of 32), fp32 accumulation per (kind, class) plus a per-mask max
     for validation; host applies empty-mask guards + mean.

VALIDATION: acc >= true EDT^2 always (windowing only drops candidates;
corrupted row distances are >= PAD > R).  If max(acc per live mask) <= R^2,
every pixel's optimal candidate (|di| <= dist <= R, row dist <= R < PAD) was
in the window with its exact value, so acc == EDT^2 exactly.  Otherwise the
host escalates to an R=15 build, then to an exact numpy fallback.

Sharding: data-parallel over batch, one sample per core; per-partition
partial sums are combined on the host (scalar loss; no collectives).
"""

import numpy as np

B, C, H, W = 8, 4, 256, 256
NCORES = 8
P = 128
INF = 4096.0
R_PRED, R_TGT = 4, 5     # primary window radii (validated on device)
R_MAX = 15               # escalation build; beyond that -> exact numpy

_CACHE = {}
LAST_RESULT = None        # raw per-core outputs of the most recent device run
LAST_EXEC_WALL_NS = None  # wall-clock of the device dispatch+exec call


def _seg(k, t, c, j):
    """Segment index: kind, polarity, class, chunk (h in row dom, v in col)."""
    return ((k * 2 + t) * 4 + c) * 2 + j


# --------------------------------------------------------------- bass kernel

def _build(R_pred, R_tgt, pad):
    import concourse.bacc as bacc
    import concourse.mybir as mybir
    from concourse.tile import TileContext

    dt = mybir.dt
    op = mybir.AluOpType
    seg = pad + W + pad
    free = 32 * seg
    Rk = {0: R_pred, 1: R_tgt}

    nc = bacc.Bacc("TRN2", target_bir_lowering=False, debug=False,
                   enable_asserts=False, num_devices=NCORES)
    xb = nc.dram_tensor("x", [C, H, W], dt.float16, kind="ExternalInput")
    yb = nc.dram_tensor("y", [H, W], dt.int32, kind="ExternalInput")
    eb = nc.dram_tensor("eye", [P, P], dt.bfloat16, kind="ExternalInput")
    ob = nc.dram_tensor("out", [P, 16], dt.float32, kind="ExternalOutput")

    with TileContext(nc) as tc:
        with tc.tile_pool(name="main", bufs=1) as pool, \
             tc.tile_pool(name="ps", bufs=8, space="PSUM") as psp:
            x_sb = pool.tile([P, C * 2 * W], dt.float16, tag="x_sb")
            y_sb = pool.tile([P, 2 * W], dt.int32, tag="y_sb")
            eye = pool.tile([P, P], dt.bfloat16, tag="eye")
            m_tgt = pool.tile([P, C * 2 * W], dt.bfloat16, tag="m_tgt")
            a = pool.tile([P, free], dt.bfloat16, tag="a")
            f = pool.tile([P, free], dt.bfloat16, tag="f")
            ones = pool.tile([P, free], dt.bfloat16, tag="ones")
            gT = pool.tile([P, free], dt.bfloat16, tag="gT")
            gT1 = pool.tile([P, free], dt.bfloat16, tag="gT1")
            acc = pool.tile([P, 32 * W], dt.bfloat16, tag="acc")
            tdiff = pool.tile([P, C * 2 * W], dt.bfloat16, tag="tdiff")
            wsqT = pool.tile([P, C * 2 * W], dt.bfloat16, tag="wsqT")
            viol = pool.tile([P, 8 * W], dt.bfloat16, tag="viol")
            prod = pool.tile([P, 2 * W], dt.float32, tag="prod")
            fin = pool.tile([P, 16], dt.float32, tag="fin")

            # ---- loads
            nc.sync.dma_start(
                out=x_sb[:, :].rearrange("p (c hh w) -> p c hh w", c=C, hh=2),
                in_=xb.ap().rearrange("c (hh p) w -> p c hh w", p=P))
            nc.sync.dma_start(
                out=y_sb[:, :].rearrange("p (hh w) -> p hh w", hh=2),
                in_=yb.ap().rearrange("(hh p) w -> p hh w", p=P))
            nc.sync.dma_start(out=eye[:, :], in_=eb.ap())

            nc.vector.memset(a[:, :], INF)
            nc.vector.memset(ones[:, :], 1.0)
            nc.vector.memset(gT[:, :], INF)  # squared-domain pad value

            # ---- target one-hot masks (bf16 0/1), layout matches x_sb
            for c in range(C):
                nc.vector.tensor_scalar(
                    out=m_tgt[:, c * 2 * W:(c + 1) * 2 * W],
                    in0=y_sb[:, :], scalar1=float(c), scalar2=None,
                    op0=op.is_equal)

            def live8(tile, k, t):
                """[p, 8, W] live columns of the 8 row segments of (k, t)."""
                base = _seg(k, t, 0, 0) * seg
                v = tile[:, base:base + 8 * seg]
                v = v.rearrange("p (s w) -> p s w", w=seg)
                return v[:, :, pad:pad + W]

            x8 = x_sb[:, :].rearrange("p (s w) -> p s w", w=W)
            m8 = m_tgt[:, :].rearrange("p (s w) -> p s w", w=W)

            # ---- scan input a: 0 at zero-set pixels, INF elsewhere
            nc.vector.tensor_scalar(out=live8(a, 0, 0), in0=x8, scalar1=0.5,
                                    scalar2=INF, op0=op.is_le, op1=op.mult)
            nc.vector.tensor_scalar(out=live8(a, 0, 1), in0=x8, scalar1=0.5,
                                    scalar2=INF, op0=op.is_gt, op1=op.mult)
            nc.vector.tensor_scalar(out=live8(a, 1, 0), in0=m8, scalar1=-INF,
                                    scalar2=INF, op0=op.mult, op1=op.add)
            nc.vector.tensor_scalar(out=live8(a, 1, 1), in0=m8, scalar1=INF,
                                    scalar2=None, op0=op.mult)

            # ---- row pass: two chained min-plus scans per (kind, pol) block
            for q in range(4):
                s0, s1 = q * 8 * seg, (q + 1) * 8 * seg
                nc.vector.tensor_tensor_scan(
                    out=f[:, s0:s1], data0=ones[:, s0:s1], data1=a[:, s0:s1],
                    initial=INF, op0=op.add, op1=op.min)
                nc.vector.tensor_tensor_scan(
                    out=a[:, s0:s1][:, ::-1], data0=ones[:, s0:s1][:, ::-1],
                    data1=f[:, s0:s1][:, ::-1],
                    initial=INF, op0=op.add, op1=op.min)

            # ---- transpose d1 via PE; square on the PSUM->SBUF drain
            for k in range(2):
                for t in range(2):
                    for c in range(C):
                        for h in range(2):
                            for v in range(2):
                                ps = psp.tile([P, P], dt.bfloat16,
                                              name="ps", tag="ps")
                                sb = _seg(k, t, c, h) * seg + pad + 128 * v
                                nc.tensor.transpose(
                                    ps[:, :], a[:, sb:sb + 128], eye[:, :])
                                db = _seg(k, t, c, v) * seg + pad + 128 * h
                                nc.scalar.square(out=gT[:, db:db + 128],
                                                 in_=ps[:, :])
                # odd-shift alias for DVE 2x alignment (scalar engine copy)
                kb, ke = k * 16 * seg, (k + 1) * 16 * seg
                nc.scalar.copy(out=gT1[:, kb:ke - 1], in_=gT[:, kb + 1:ke])

            # ---- column pass: acc = min_d ( g[i +- d] + d^2 ), d = 0..Rk
            for k in range(2):
                accv = acc[:, k * 16 * W:(k + 1) * 16 * W]
                accv = accv.rearrange("p (s w) -> p s w", w=W)

                def gview(tile, shift, k=k):
                    vv = tile[:, k * 16 * seg:(k + 1) * 16 * seg]
                    vv = vv.rearrange("p (s w) -> p s w", w=seg)
                    return vv[:, :, pad + shift:pad + shift + W]

                first = True
                ds = [d for d in range(2, Rk[k] + 1, 2)] + \
                     [d for d in range(1, Rk[k] + 1, 2)]
                for d in ds:
                    for sgn in (1, -1):
                        if d % 2 == 0:
                            in0 = gview(gT, sgn * d)
                        else:
                            in0 = gview(gT1, sgn * d - 1)
                        in1 = gview(gT, 0) if first else accv
                        nc.vector.scalar_tensor_tensor(
                            out=accv, in0=in0, scalar=float(d * d), in1=in1,
                            op0=op.add, op1=op.min)
                        first = False

            # ---- S = edt2(m) + edt2(~m): pol T += pol F (into T block)
            for k in range(2):
                t0 = acc[:, (k * 16) * W:(k * 16 + 8) * W]
                t1 = acc[:, (k * 16 + 8) * W:(k * 16 + 16) * W]
                nc.vector.tensor_add(out=t0, in0=t0, in1=t1)

            # ---- weights: (x - onehot)^2, transposed via PE + squared drain
            nc.vector.tensor_sub(out=tdiff[:, :], in0=x_sb[:, :],
                                 in1=m_tgt[:, :])
            for c in range(C):
                for h in range(2):
                    for v in range(2):
                        ps = psp.tile([P, P], dt.bfloat16,
                                      name="ps", tag="ps")
                        sb = (c * 2 + h) * W + 128 * v
                        nc.tensor.transpose(
                            ps[:, :], tdiff[:, sb:sb + 128], eye[:, :])
                        db = (c * 2 + v) * W + 128 * h
                        nc.scalar.square(out=wsqT[:, db:db + 128],
                                         in_=ps[:, :])

            # ---- per-(kind,class) weighted sums + validation counts
            # (tensor_tensor_reduce and max-reduce crash this HW's ucode, so
            # use mult + add-reduce and an is_gt violation count instead)
            for k in range(2):
                merged = acc[:, (k * 16) * W:(k * 16 + 8) * W]
                nc.vector.tensor_scalar(
                    out=viol[:, :], in0=merged,
                    scalar1=float(Rk[k] * Rk[k]) + 0.5, scalar2=None,
                    op0=op.is_gt)
                for c in range(C):
                    i = k * 4 + c
                    accs = acc[:, (k * 16 + c * 2) * W:(k * 16 + c * 2 + 2) * W]
                    nc.vector.tensor_mul(
                        out=prod[:, :],
                        in0=wsqT[:, c * 2 * W:(c + 1) * 2 * W], in1=accs)
                    nc.vector.tensor_reduce(
                        out=fin[:, i:i + 1], in_=prod[:, :],
                        axis=mybir.AxisListType.X, op=op.add)
                    nc.vector.tensor_reduce(
                        out=fin[:, 8 + i:9 + i],
                        in_=viol[:, c * 2 * W:(c + 1) * 2 * W],
                        axis=mybir.AxisListType.X, op=op.add)

            nc.sync.dma_start(out=ob.ap(), in_=fin[:, :])

    nc.compile()
    return nc


def _get_nc(key):
    if key not in _CACHE:
        if key == "main":
            _CACHE[key] = _build(R_PRED, R_TGT, 8)
        else:
            _CACHE[key] = _build(R_MAX, R_MAX, 16)
    return _CACHE[key]


def _run_device(nc, x16, y):
    """Run the 8-core SPMD kernel; returns (sums[B,2,C], maxes[B,2,C])."""
    global LAST_RESULT, LAST_EXEC_WALL_NS
    import time
    import ml_dtypes
    from concourse import bass2jax

    eye = np.eye(P, dtype=ml_dtypes.bfloat16)
    in_maps = [{"x": x16[b], "y": y[b], "eye": eye} for b in range(B)]
    t0 = time.perf_counter()
    res = bass2jax.run_bass_via_pjrt(nc, in_maps, n_cores=NCORES)
    LAST_EXEC_WALL_NS = int((time.perf_counter() - t0) * 1e9)
    LAST_RESULT = res
    out = np.stack([res[b]["out"] for b in range(B)])      # [B,128,16]
    sums = out[:, :, :8].astype(np.float64).sum(axis=1).reshape(B, 2, C)
    viols = out[:, :, 8:].sum(axis=1).reshape(B, 2, C)
    return sums, viols


# ------------------------------------------------------- exact host fallback

def _loss_numpy_exact(x, y):
    BIG = float(H + W)

    def dist1d(z):
        n = z.shape[-1]
        idx = np.arange(n, dtype=np.int64)
        fw = np.maximum.accumulate(np.where(z, idx, -1), axis=-1)
        df = np.where(fw >= 0, (idx - fw).astype(np.float32), np.float32(BIG))
        bw = np.minimum.accumulate(np.where(z, idx, 2 * n)[..., ::-1],
                                   axis=-1)[..., ::-1]
        db = np.where(bw < 2 * n, (bw - idx).astype(np.float32),
                      np.float32(BIG))
        return np.minimum(df, db)

    def edt_sq(z):
        g = dist1d(z).astype(np.float32) ** 2
        i = np.arange(H, dtype=np.float32)
        out = np.empty((H, W), np.float32)
        for i0 in range(0, H, 32):
            off = (i[i0:i0 + 32, None] - i[None, :]) ** 2
            out[i0:i0 + 32] = (off[:, :, None] + g[None, :, :]).min(axis=1)
        return out

    def field(m):
        if not m.any():
            return np.zeros((H, W), np.float32)
        return np.sqrt(edt_sq(~m)) + np.sqrt(edt_sq(m))

    total = 0.0
    for b in range(B):
        for c in range(C):
            oh = (y[b] == c)
            pm = x[b, c] > 0.5
            dist = (field(pm).astype(np.float32) ** 2
                    + field(oh).astype(np.float32) ** 2)
            w = (x[b, c] - oh.astype(np.float32)) ** 2
            total += float((w.astype(np.float64) * dist.astype(np.float64)).sum())
    return np.float32(total / (B * C * H * W))


# ------------------------------------------------------------------- driver

def kernel(x, y):
    x = np.asarray(x, np.float32)
    y = np.ascontiguousarray(np.asarray(y, np.int32))
    assert x.shape == (B, C, H, W) and y.shape == (B, H, W)
    x16 = np.ascontiguousarray(x.astype(np.float16))

    # empty-mask guards (reference zeroes the field of an empty mask)
    g_pred = (x > 0.5).reshape(B, C, -1).any(axis=2)
    g_tgt = np.stack([(y == c).reshape(B, -1).any(axis=1) for c in range(C)],
                     axis=1)
    guards = np.stack([g_pred, g_tgt], axis=1)             # [B,2,C]

    try:
        for key in ("main", "esc"):
            nc = _get_nc(key)
            sums, viols = _run_device(nc, x16, y)
            if (viols[guards] != 0).any():
                continue  # window too small for some live mask -> escalate
            total = float((sums * guards).sum())
            return np.asarray(np.float32(total / (B * C * H * W)))
    except Exception as e:  # device unavailable etc. -> exact host fallback
        import sys
        print(f"kernel: device path failed ({type(e).__name__}: {e}); "
              "using exact host fallback", file=sys.stderr)
    return np.asarray(_loss_numpy_exact(x, y))


# revision 6
# speedup vs baseline: 185.4212x; 1.6037x over previous
"""HausdorffDT loss kernel for Trainium2 (8 NeuronCores, Bass/Tile).

Math: with ALPHA=2 and field(m) = sqrt(edt2(m)) + sqrt(edt2(~m)), one of the
two terms is zero at every pixel, so field(m)^2 == edt2(m) + edt2(~m) exactly.
The loss is therefore

    mean( (x - onehot)^2 * (edt2(pm)+edt2(~pm) + edt2(tm)+edt2(~tm)) )

with an all-zero-field guard per empty mask.  Squared EDTs are small exact
integers, so the distance pipeline runs in bf16 exactly:

  1. row pass: exact 1D distance to nearest True along W via two chained
     tensor_tensor_scan min-plus recurrences (fwd + bwd), batched over all
     fields with INF padding between row segments.
  2. transpose the row distances via the (otherwise idle) tensor engine,
     squaring them in the PSUM->SBUF drain on the scalar engine (fused).
  3. column pass: windowed parabola min-plus
     acc = min(acc, g[i +- d] + d^2), d = 1..R.  Exact whenever the true max
     distance of the mask is <= R, which is validated on-device: row pads of
     width PAD with PAD^2 > R^2 guarantee every corrupted/leaked row distance
     contributes candidates > R^2, so  max(final acc) <= R^2  iff the result
     is the exact EDT^2 (see VALIDATION below).
  4. weighted reduce against (x - onehot)^2 directly in the transposed
     domain (the weights are transposed instead of the fields: 16 tiles
     instead of 32), fp32 accumulation per (kind, class) plus a per-mask max
     for validation; host applies empty-mask guards + mean.

VALIDATION: acc >= true EDT^2 always (windowing only drops candidates;
corrupted row distances are >= PAD > R).  If max(acc per live mask) <= R^2,
every pixel's optimal candidate (|di| <= dist <= R, row dist <= R < PAD) was
in the window with its exact value, so acc == EDT^2 exactly.  Otherwise the
host escalates to an R=15 build, then to an exact numpy fallback.

Sharding: data-parallel over batch, one sample per core; per-partition
partial sums are combined on the host (scalar loss; no collectives).
"""

import numpy as np

B, C, H, W = 8, 4, 256, 256
NCORES = 8
P = 128
INF = 4096.0
R_PRED, R_TGT = 4, 5     # primary window radii (validated on device)
R_MAX = 15               # escalation build; beyond that -> exact numpy

_CACHE = {}
LAST_RESULT = None        # raw per-core outputs of the most recent device run
LAST_EXEC_WALL_NS = None  # wall-clock of the device dispatch+exec call


def _seg(k, t, c, j):
    """Segment index: kind, polarity, class, chunk (h in row dom, v in col)."""
    return ((k * 2 + t) * 4 + c) * 2 + j


# --------------------------------------------------------------- bass kernel

def _build(R_pred, R_tgt, pad):
    import concourse.bacc as bacc
    import concourse.mybir as mybir
    from concourse.tile import TileContext

    dt = mybir.dt
    op = mybir.AluOpType
    seg = pad + W + pad
    free = 32 * seg
    Rk = {0: R_pred, 1: R_tgt}

    nc = bacc.Bacc("TRN2", target_bir_lowering=False, debug=False,
                   enable_asserts=False, num_devices=NCORES)
    xb = nc.dram_tensor("x", [C, H, W], dt.float16, kind="ExternalInput")
    yb = nc.dram_tensor("y", [H, W], dt.int32, kind="ExternalInput")
    eb = nc.dram_tensor("eye", [P, P], dt.bfloat16, kind="ExternalInput")
    ob = nc.dram_tensor("out", [P, 16], dt.float32, kind="ExternalOutput")

    with TileContext(nc) as tc:
        with tc.tile_pool(name="main", bufs=1) as pool, \
             tc.tile_pool(name="ps", bufs=4, space="PSUM") as psp:
            x_sb = pool.tile([P, C * 2 * W], dt.float16, tag="x_sb")
            y_sb = pool.tile([P, 2 * W], dt.int32, tag="y_sb")
            eye = pool.tile([P, P], dt.bfloat16, tag="eye")
            m_tgt = pool.tile([P, C * 2 * W], dt.bfloat16, tag="m_tgt")
            a = pool.tile([P, free], dt.bfloat16, tag="a")
            f = pool.tile([P, free], dt.bfloat16, tag="f")
            ones = pool.tile([P, free], dt.bfloat16, tag="ones")
            gT = pool.tile([P, free], dt.bfloat16, tag="gT")
            gT1 = pool.tile([P, free], dt.bfloat16, tag="gT1")
            acc = pool.tile([P, 32 * W], dt.bfloat16, tag="acc")
            tdiff = pool.tile([P, C * 2 * W], dt.bfloat16, tag="tdiff")
            wsqT = pool.tile([P, C * 2 * W], dt.bfloat16, tag="wsqT")
            viol = pool.tile([P, 8 * W], dt.bfloat16, tag="viol")
            prod = pool.tile([P, 2 * W], dt.float32, tag="prod")
            fin = pool.tile([P, 16], dt.float32, tag="fin")

            # ---- loads
            nc.sync.dma_start(
                out=x_sb[:, :].rearrange("p (c hh w) -> p c hh w", c=C, hh=2),
                in_=xb.ap().rearrange("c (hh p) w -> p c hh w", p=P))
            nc.sync.dma_start(
                out=y_sb[:, :].rearrange("p (hh w) -> p hh w", hh=2),
                in_=yb.ap().rearrange("(hh p) w -> p hh w", p=P))
            nc.sync.dma_start(out=eye[:, :], in_=eb.ap())

            nc.vector.memset(a[:, :], INF)
            nc.vector.memset(ones[:, :], 1.0)
            nc.vector.memset(gT[:, :], INF)  # squared-domain pad value

            # ---- target one-hot masks (bf16 0/1), layout matches x_sb
            for c in range(C):
                nc.vector.tensor_scalar(
                    out=m_tgt[:, c * 2 * W:(c + 1) * 2 * W],
                    in0=y_sb[:, :], scalar1=float(c), scalar2=None,
                    op0=op.is_equal)

            def live8(tile, k, t):
                """[p, 8, W] live columns of the 8 row segments of (k, t)."""
                base = _seg(k, t, 0, 0) * seg
                v = tile[:, base:base + 8 * seg]
                v = v.rearrange("p (s w) -> p s w", w=seg)
                return v[:, :, pad:pad + W]

            x8 = x_sb[:, :].rearrange("p (s w) -> p s w", w=W)
            m8 = m_tgt[:, :].rearrange("p (s w) -> p s w", w=W)

            # ---- scan input a: 0 at zero-set pixels, INF elsewhere
            nc.vector.tensor_scalar(out=live8(a, 0, 0), in0=x8, scalar1=0.5,
                                    scalar2=INF, op0=op.is_le, op1=op.mult)
            nc.vector.tensor_scalar(out=live8(a, 0, 1), in0=x8, scalar1=0.5,
                                    scalar2=INF, op0=op.is_gt, op1=op.mult)
            nc.vector.tensor_scalar(out=live8(a, 1, 0), in0=m8, scalar1=-INF,
                                    scalar2=INF, op0=op.mult, op1=op.add)
            nc.vector.tensor_scalar(out=live8(a, 1, 1), in0=m8, scalar1=INF,
                                    scalar2=None, op0=op.mult)

            # ---- row pass: two chained min-plus scans per (kind, pol) block
            for q in range(4):
                s0, s1 = q * 8 * seg, (q + 1) * 8 * seg
                nc.vector.tensor_tensor_scan(
                    out=f[:, s0:s1], data0=ones[:, s0:s1], data1=a[:, s0:s1],
                    initial=INF, op0=op.add, op1=op.min)
                nc.vector.tensor_tensor_scan(
                    out=a[:, s0:s1][:, ::-1], data0=ones[:, s0:s1][:, ::-1],
                    data1=f[:, s0:s1][:, ::-1],
                    initial=INF, op0=op.add, op1=op.min)

            # ---- transpose d1 via PE; square on the PSUM->SBUF drain.
            # 4 tile-transposes (h,v) share one PSUM bank, drained by a
            # single scalar-engine activation per (k,t,c) group.
            for k in range(2):
                for t in range(2):
                    for c in range(C):
                        ps = psp.tile([P, 4 * P], dt.bfloat16,
                                      name="ps", tag="ps")
                        for h in range(2):
                            for v in range(2):
                                sb = _seg(k, t, c, h) * seg + pad + 128 * v
                                nc.tensor.transpose(
                                    ps[:, (v * 2 + h) * P:(v * 2 + h + 1) * P],
                                    a[:, sb:sb + 128], eye[:, :])
                        base = _seg(k, t, c, 0) * seg
                        dv = gT[:, base:base + 2 * seg]
                        dv = dv.rearrange("p (v s) -> p v s", v=2)
                        nc.scalar.square(
                            out=dv[:, :, pad:pad + W],
                            in_=ps[:, :].rearrange("p (v s) -> p v s", v=2))
                # odd-shift alias for DVE 2x alignment (scalar engine copy)
                kb, ke = k * 16 * seg, (k + 1) * 16 * seg
                nc.scalar.copy(out=gT1[:, kb:ke - 1], in_=gT[:, kb + 1:ke])

            # ---- column pass: acc = min_d ( g[i +- d] + d^2 ), d = 0..Rk
            for k in range(2):
                accv = acc[:, k * 16 * W:(k + 1) * 16 * W]
                accv = accv.rearrange("p (s w) -> p s w", w=W)

                def gview(tile, shift, k=k):
                    vv = tile[:, k * 16 * seg:(k + 1) * 16 * seg]
                    vv = vv.rearrange("p (s w) -> p s w", w=seg)
                    return vv[:, :, pad + shift:pad + shift + W]

                first = True
                ds = [d for d in range(2, Rk[k] + 1, 2)] + \
                     [d for d in range(1, Rk[k] + 1, 2)]
                for d in ds:
                    for sgn in (1, -1):
                        if d % 2 == 0:
                            in0 = gview(gT, sgn * d)
                        else:
                            in0 = gview(gT1, sgn * d - 1)
                        in1 = gview(gT, 0) if first else accv
                        nc.vector.scalar_tensor_tensor(
                            out=accv, in0=in0, scalar=float(d * d), in1=in1,
                            op0=op.add, op1=op.min)
                        first = False

            # ---- S = edt2(m) + edt2(~m): pol T += pol F (into T block)
            for k in range(2):
                t0 = acc[:, (k * 16) * W:(k * 16 + 8) * W]
                t1 = acc[:, (k * 16 + 8) * W:(k * 16 + 16) * W]
                nc.vector.tensor_add(out=t0, in0=t0, in1=t1)

            # ---- weights: (x - onehot)^2, transposed via PE + squared drain
            nc.vector.tensor_sub(out=tdiff[:, :], in0=x_sb[:, :],
                                 in1=m_tgt[:, :])
            for c in range(C):
                ps = psp.tile([P, 4 * P], dt.bfloat16, name="ps", tag="ps")
                for h in range(2):
                    for v in range(2):
                        sb = (c * 2 + h) * W + 128 * v
                        nc.tensor.transpose(
                            ps[:, (v * 2 + h) * P:(v * 2 + h + 1) * P],
                            tdiff[:, sb:sb + 128], eye[:, :])
                nc.scalar.square(out=wsqT[:, c * 2 * W:(c + 1) * 2 * W],
                                 in_=ps[:, :])

            # ---- per-(kind,class) weighted sums + validation counts
            # (tensor_tensor_reduce and max-reduce crash this HW's ucode, so
            # use mult + add-reduce and an is_gt violation count instead)
            for k in range(2):
                merged = acc[:, (k * 16) * W:(k * 16 + 8) * W]
                nc.vector.tensor_scalar(
                    out=viol[:, :], in0=merged,
                    scalar1=float(Rk[k] * Rk[k]) + 0.5, scalar2=None,
                    op0=op.is_gt)
                for c in range(C):
                    i = k * 4 + c
                    accs = acc[:, (k * 16 + c * 2) * W:(k * 16 + c * 2 + 2) * W]
                    nc.vector.tensor_mul(
                        out=prod[:, :],
                        in0=wsqT[:, c * 2 * W:(c + 1) * 2 * W], in1=accs)
                    nc.vector.tensor_reduce(
                        out=fin[:, i:i + 1], in_=prod[:, :],
                        axis=mybir.AxisListType.X, op=op.add)
                    nc.vector.tensor_reduce(
                        out=fin[:, 8 + i:9 + i],
                        in_=viol[:, c * 2 * W:(c + 1) * 2 * W],
                        axis=mybir.AxisListType.X, op=op.add)

            nc.sync.dma_start(out=ob.ap(), in_=fin[:, :])

    nc.compile()
    return nc


def _make_runner(nc):
    """jit-compile the 8-core SPMD dispatch once; returns a closure."""
    import jax
    import ml_dtypes
    from jax.sharding import Mesh, PartitionSpec
    from jax.experimental.shard_map import shard_map
    import concourse.mybir as mybir
    from concourse.bass2jax import (_bass_exec_p, partition_id_tensor,
                                    install_neuronx_cc_hook)

    try:  # persist XLA executables (incl. the wrapped NEFF) across processes
        jax.config.update("jax_compilation_cache_dir", "/tmp/jax_cc")
        jax.config.update("jax_persistent_cache_min_entry_size_bytes", 0)
        jax.config.update("jax_persistent_cache_min_compile_time_secs", 0.0)
    except Exception:
        pass
    install_neuronx_cc_hook()

    partition_name = (nc.partition_id_tensor.name
                      if nc.partition_id_tensor else None)
    in_names, out_names, out_avals, zero_outs = [], [], [], []
    for alloc in nc.m.functions[0].allocations:
        if not isinstance(alloc, mybir.MemoryLocationSet):
            continue
        name = alloc.memorylocations[0].name
        if alloc.kind == "ExternalInput":
            if name != partition_name:
                in_names.append(name)
        elif alloc.kind == "ExternalOutput":
            shape = tuple(alloc.tensor_shape)
            dtype = mybir.dt.np(alloc.dtype)
            out_avals.append(jax.core.ShapedArray(shape, dtype))
            out_names.append(name)
            zero_outs.append(np.zeros(shape, dtype))
    n_params = len(in_names)
    in_names = in_names + out_names
    if partition_name is not None:
        in_names.append(partition_name)
    donate = tuple(range(n_params, n_params + len(out_names)))

    def _body(*args):
        operands = list(args)
        if partition_name is not None:
            operands.append(partition_id_tensor())
        outs = _bass_exec_p.bind(
            *operands, out_avals=tuple(out_avals),
            in_names=tuple(in_names), out_names=tuple(out_names),
            lowering_input_output_aliases=(),
            sim_require_finite=True, sim_require_nnan=True, nc=nc)
        return tuple(outs)

    devices = jax.devices()[:NCORES]
    mesh = Mesh(np.asarray(devices), ("core",))
    in_specs = (PartitionSpec("core"),) * (n_params + len(out_names))
    out_specs = (PartitionSpec("core"),) * len(out_names)
    sharded = jax.jit(shard_map(_body, mesh=mesh, in_specs=in_specs,
                                out_specs=out_specs, check_rep=False),
                      donate_argnums=donate, keep_unused=True)
    eye = np.eye(P, dtype=ml_dtypes.bfloat16)
    eye8 = np.concatenate([eye] * NCORES, axis=0)
    order = {n: i for i, n in enumerate(in_names[:n_params])}

    def run(x16, y):
        ins = [None] * n_params
        ins[order["x"]] = x16.reshape(B * C, H, W)
        ins[order["y"]] = y.reshape(B * H, W)
        ins[order["eye"]] = eye8
        zeros = [np.zeros((NCORES * z.shape[0], *z.shape[1:]), z.dtype)
                 for z in zero_outs]
        outs = sharded(*ins, *zeros)
        return np.asarray(outs[0]).reshape(NCORES, P, 16)

    return run


def _get_runner(key):
    if key not in _CACHE:
        nc = (_build(R_PRED, R_TGT, 8) if key == "main"
              else _build(R_MAX, R_MAX, 16))
        _CACHE[key] = _make_runner(nc)
    return _CACHE[key]


def _run_device(run, x16, y):
    """Run the 8-core SPMD kernel; returns (sums[B,2,C], viols[B,2,C])."""
    global LAST_RESULT, LAST_EXEC_WALL_NS
    import time
    t0 = time.perf_counter()
    out = run(x16, y)                                      # [B,128,16]
    LAST_EXEC_WALL_NS = int((time.perf_counter() - t0) * 1e9)
    LAST_RESULT = out
    sums = out[:, :, :8].astype(np.float64).sum(axis=1).reshape(B, 2, C)
    viols = out[:, :, 8:].sum(axis=1).reshape(B, 2, C)
    return sums, viols


# ------------------------------------------------------- exact host fallback

def _loss_numpy_exact(x, y):
    BIG = float(H + W)

    def dist1d(z):
        n = z.shape[-1]
        idx = np.arange(n, dtype=np.int64)
        fw = np.maximum.accumulate(np.where(z, idx, -1), axis=-1)
        df = np.where(fw >= 0, (idx - fw).astype(np.float32), np.float32(BIG))
        bw = np.minimum.accumulate(np.where(z, idx, 2 * n)[..., ::-1],
                                   axis=-1)[..., ::-1]
        db = np.where(bw < 2 * n, (bw - idx).astype(np.float32),
                      np.float32(BIG))
        return np.minimum(df, db)

    def edt_sq(z):
        g = dist1d(z).astype(np.float32) ** 2
        i = np.arange(H, dtype=np.float32)
        out = np.empty((H, W), np.float32)
        for i0 in range(0, H, 32):
            off = (i[i0:i0 + 32, None] - i[None, :]) ** 2
            out[i0:i0 + 32] = (off[:, :, None] + g[None, :, :]).min(axis=1)
        return out

    def field(m):
        if not m.any():
            return np.zeros((H, W), np.float32)
        return np.sqrt(edt_sq(~m)) + np.sqrt(edt_sq(m))

    total = 0.0
    for b in range(B):
        for c in range(C):
            oh = (y[b] == c)
            pm = x[b, c] > 0.5
            dist = (field(pm).astype(np.float32) ** 2
                    + field(oh).astype(np.float32) ** 2)
            w = (x[b, c] - oh.astype(np.float32)) ** 2
            total += float((w.astype(np.float64) * dist.astype(np.float64)).sum())
    return np.float32(total / (B * C * H * W))


# ------------------------------------------------------------------- driver

def kernel(x, y):
    x = np.asarray(x, np.float32)
    y = np.ascontiguousarray(np.asarray(y, np.int32))
    assert x.shape == (B, C, H, W) and y.shape == (B, H, W)
    x16 = np.ascontiguousarray(x.astype(np.float16))

    # empty-mask guards (reference zeroes the field of an empty mask)
    g_pred = (x > 0.5).reshape(B, C, -1).any(axis=2)
    g_tgt = np.stack([(y == c).reshape(B, -1).any(axis=1) for c in range(C)],
                     axis=1)
    guards = np.stack([g_pred, g_tgt], axis=1)             # [B,2,C]

    try:
        for key in ("main", "esc"):
            run = _get_runner(key)
            sums, viols = _run_device(run, x16, y)
            if (viols[guards] != 0).any():
                continue  # window too small for some live mask -> escalate
            total = float((sums * guards).sum())
            return np.asarray(np.float32(total / (B * C * H * W)))
    except Exception as e:  # device unavailable etc. -> exact host fallback
        import sys
        print(f"kernel: device path failed ({type(e).__name__}: {e}); "
              "using exact host fallback", file=sys.stderr)
    return np.asarray(_loss_numpy_exact(x, y))
